# revision 3
# baseline (speedup 1.0000x reference)
"""Trainium2 Bass kernel for a 6-layer binary CNN (XNOR-net style), v2.

Contract: kernel(**inputs) takes the FULL unsharded inputs (batch 128) and
returns the FULL output [128, 4, 4, 10] float32.

Strategy (v2 changes over the 191.5us baseline)
-----------------------------------------------
1. conv1 in fp16 hi/lo 3-term split (one K=81 pass instead of fp32's
   ~4 cycles/col): rows [x_hi; x_lo; x_hi] vs [w_hi; w_hi; w_lo], fp32-grade
   accuracy (dropped term ~2^-22).  conv1 MMs interleave image-by-image with
   L2's so the PE never waits on the xcol DMA.
2. Sign trick: sign(s*relu(z)+b) == sign(s*z+b) when b<0, and == +1 const
   when b>0 (emit scale=0, bias=1).  Kills every standalone DVE relu; ACT
   reads PSUM directly.
3. Dead-channel collapse at the bn5 boundary: only 247 of 512 bn5 channels
   have b<0; the 265 dead ones are all the constant +1 plane, folded into ONE
   representative ones-plane whose L6 weight row is the summed dead weights.
   -> L5 emits 2 output groups instead of 4; L6 runs 9 DR passes instead of
   18.  (bn1..bn4 boundaries miss their 128/256 cliffs; left unpacked.)
4. bn6 folds into the dense weights (dwp' = s6*dw, db' = db + b6@dw); dense
   matmuls accumulate per-go-group as h6 chunks complete (shorter tail).
5. 12 warmup matmuls on a zeroed fp16 tile run during the input DMA window
   so HAM reaches K=8/8 before real work.

Binary-layer arithmetic stays exact (+-1 products in fp32 PSUM).
"""

import numpy as np
import ml_dtypes

_F8 = ml_dtypes.float8_e4m3
_F16 = np.float16

B = 16        # images per core
N_CORES = 8

_WCOLS = {2: 1280, 3: 2560, 4: 4608, 5: 4608, 6: 9216}

# a5/a6 wide geometry: 16 images of width 8 + separators, row pitch 152,
# rows 0..9 (8 interior), 8-elem left guard, right pad to grp stride 1552.
_WP5 = 152
_G5 = 8
_S5 = 1552

_prog_cache = {}


def _build_program(n_alive5):
    """n_alive5: number of bn5-alive channels (b<0).  a6 carries
    n_alive5 + 1 (ones-repr) packed channels."""
    key = ("v2", n_alive5)
    if key in _prog_cache:
        return _prog_cache[key]

    from contextlib import ExitStack

    import concourse.bacc as bacc
    import concourse.mybir as mybir
    import concourse.tile as tile
    from concourse.ap import AP

    dt = mybir.dt
    AL = mybir.AluOpType
    AF = mybir.ActivationFunctionType
    AX = mybir.AxisListType
    DR = mybir.MatmulPerfMode.DoubleRow

    nc = bacc.Bacc("TRN2", target_bir_lowering=False, debug=False,
                   num_devices=N_CORES)

    f32 = dt.float32
    f8 = dt.float8e4
    f16 = dt.float16

    na5 = n_alive5            # alive bn5 channels
    # a6 group-1 real channels: alive tail + TWO ones-repr planes carrying
    # the dead-channel weight mass as r + 16q (both fp8-exact; one fp8 row
    # of the raw sum would round: e4m3 integers are exact only to +-16)
    ng1 = na5 + 2 - 128
    assert 1 < ng1 <= 128

    d_xcol = nc.dram_tensor("xcol", [81, 16384], f16, kind="ExternalInput").ap()
    d_w1p = nc.dram_tensor("w1p", [81, 128], f16, kind="ExternalInput").ap()
    d_bnv = nc.dram_tensor("bnv", [128, 16], f32, kind="ExternalInput").ap()
    d_dwp = nc.dram_tensor("dwp", [128, 40], f32, kind="ExternalInput").ap()
    d_db = nc.dram_tensor("db", [1, 10], f32, kind="ExternalInput").ap()
    d_w = {l: nc.dram_tensor(f"wb{l}", [128, _WCOLS[l]], f8,
                             kind="ExternalInput").ap()
           for l in (2, 3, 4, 5, 6)}
    d_out = nc.dram_tensor("out", [256, 10], f32, kind="ExternalOutput").ap()

    with tile.TileContext(nc) as tc, ExitStack() as ctx:
        consts = ctx.enter_context(tc.tile_pool(name="consts", bufs=1))
        psum_pool = ctx.enter_context(
            tc.tile_pool(name="cpsum", bufs=6, space="PSUM"))
        psum_d = ctx.enter_context(
            tc.tile_pool(name="dpsum", bufs=2, space="PSUM"))
        tmps = ctx.enter_context(tc.tile_pool(name="tmps", bufs=4))
        small = ctx.enter_context(tc.tile_pool(name="small", bufs=2))

        # ---- warmup tile + constant loads ---------------------------------
        # (memset on gpsimd: it starts ~2us before the vector engine, so the
        # PE warmup matmuls can begin that much earlier)
        wz = consts.tile([81, 640], f16, tag="wz")
        nc.gpsimd.memset(wz[:], 0.0)

        # xcol as 16 INDEPENDENT per-image tiles so each conv1 matmul
        # depends only on its own image's DMA chunk, not all 2.65MB
        w1_sb = consts.tile([81, 128], f16, tag="w1p")
        nc.sync.dma_start(w1_sb[:], d_w1p)
        xc = [consts.tile([81, 1024], f16, tag=f"xc{i}", name=f"xc{i}")
              for i in range(B)]
        for i in (0, 1):
            nc.sync.dma_start(xc[i][:], d_xcol[:, 1024 * i:1024 * (i + 1)])
        bn_sb = consts.tile([128, 16], f32, tag="bnv")
        nc.sync.dma_start(bn_sb[:], d_bnv)
        # everything else rides the sync queue, ordered by need-time.  The
        # scalar (ACT) engine issues NO DMAs: a DMA issue stalls for DGE
        # queue space, which would block the Signs queued behind it on the
        # same sequencer.
        w_sb = {}
        for l in (2, 3, 4, 5, 6):
            w_sb[l] = consts.tile([128, _WCOLS[l]], f8, tag=f"wb{l}",
                                  name=f"wb{l}")
        nc.sync.dma_start(w_sb[2][:], d_w[2])
        for i in range(2, 9):
            nc.sync.dma_start(xc[i][:], d_xcol[:, 1024 * i:1024 * (i + 1)])
        nc.sync.dma_start(w_sb[3][:], d_w[3])
        for i in range(9, 16):
            nc.sync.dma_start(xc[i][:], d_xcol[:, 1024 * i:1024 * (i + 1)])
        nc.sync.dma_start(w_sb[4][:], d_w[4])
        nc.sync.dma_start(w_sb[5][:], d_w[5])
        nc.sync.dma_start(w_sb[6][:], d_w[6])
        dwp_sb = consts.tile([128, 40], f32, tag="dwp")
        db_sb = consts.tile([1, 10], f32, tag="db")
        ones_sb = consts.tile([1, 128], f32, tag="ones")
        nc.vector.memset(ones_sb[:], 1.0)

        # ---- activation buffers (single plane: DR pairs the (1,0)+(1,1)
        # taps with an in-plane j-stride of 1 -- no shifted copies needed)
        a2 = consts.tile([128, B, 35, 40], f8, tag="a2", name="a2")
        a3 = consts.tile([128, B, 20, 24], f8, tag="a3", name="a3")
        a4 = consts.tile([128, 2, B, 18, 18], f8, tag="a4", name="a4")
        a5 = consts.tile([128, 2, _S5], f8, tag="a5", name="a5")
        a6 = consts.tile([128, 2, _S5], f8, tag="a6", name="a6")
        h6 = [consts.tile([128, B, 4, 4], f32, tag=f"h6{i}", name=f"h6{i}")
              for i in range(4)]

        # ---- PE warmup: self-contained matmuls on zeros -------------------
        for i in range(8):
            wp = psum_pool.tile([128, 16, 32], f32, tag="cps", name="cps")
            nc.tensor.matmul(wp[:].rearrange("p a b -> p (a b)"),
                             wz[:, 0:128], wz[:, 128:640],
                             start=True, stop=True)

        # halo/guard memsets (interiors are fully overwritten by ACT writes),
        # batched over half the images per op for startup pipelining
        for b0 in (0, 8):
            nc.gpsimd.memset(a2[:, b0:b0 + 8, 33:35, 0:35], 0.0)
            nc.gpsimd.memset(a2[:, b0:b0 + 8, 0, 0:35], 0.0)
            nc.gpsimd.memset(a2[:, b0:b0 + 8, 1:33, 33:35], 0.0)
            nc.gpsimd.memset(a2[:, b0:b0 + 8, 1:33, 0], 0.0)
        nc.gpsimd.memset(a3[:, :, 17:19, 0:18], 0.0)
        nc.gpsimd.memset(a3[:, :, 0, 0:18], 0.0)
        nc.gpsimd.memset(a3[:, :, 1:17, 17], 0.0)
        nc.gpsimd.memset(a3[:, :, 1:17, 0], 0.0)
        for g in range(2):
            nc.gpsimd.memset(a4[:, g, :, 0, :], 0.0)
            nc.gpsimd.memset(a4[:, g, :, 17, :], 0.0)
            nc.gpsimd.memset(a4[:, g, :, 1:17, 0], 0.0)
            nc.gpsimd.memset(a4[:, g, :, 1:17, 17], 0.0)

        def memset_wide(t):
            for g in range(2):
                # left guard + top halo row (contiguous 0..159)
                nc.gpsimd.memset(t[:, g, 0:_G5 + _WP5], 0.0)
                # bottom halo row + right guard
                nc.gpsimd.memset(t[:, g, _G5 + 9 * _WP5:_S5], 0.0)
                v = t[:, g, _G5:_G5 + 10 * _WP5].rearrange(
                    "p (r c) -> p r c", r=10)
                # image-separator columns 0,9,...,144 and right pad 145..151
                nc.gpsimd.memset(v[:, 1:9, 0:146:9], 0.0)
                nc.gpsimd.memset(v[:, 1:9, 145:152], 0.0)

        def sb(col):
            return bn_sb[:, col:col + 1]

        # ---- conv1 emission (fp16, one K=81 pass per half-image) ----------
        s1_ap, b1_ap = sb(0), sb(1)

        def conv1_emit(b):
            for h in range(2):
                pt = psum_pool.tile([128, 16, 32], f32, tag="cps", name="cps")
                rhs = xc[b][:, 512 * h:512 * (h + 1)]
                nc.tensor.matmul(pt[:].rearrange("p a b -> p (a b)"),
                                 w1_sb[:], rhs, start=True, stop=True)
                nc.scalar.activation(a2[:, b, 1 + 16 * h:17 + 16 * h, 1:33],
                                     pt[:], AF.Sign, bias=b1_ap, scale=s1_ap)

        # ---- L2 emission: tap-paired DR (5 passes), per-image halves ------
        a2_ap = a2[:]
        a2_ps = a2_ap.ap[0][0]         # partition stride
        s2_ap, b2_ap = sb(2), sb(3)

        def l2_emit(b):
            for h in range(2):
                y0 = 16 * h
                pt = psum_pool.tile([128, 16, 32], f32, tag="cps", name="cps")
                for p in range(5):
                    if p < 3:
                        off, js = b * 1400 + y0 * 40 + p, 80
                    elif p == 3:       # (1,0)+(1,1): in-plane j-stride 1
                        off, js = b * 1400 + (y0 + 1) * 40, 1
                    else:
                        off, js = b * 1400 + (y0 + 1) * 40 + 2, 80
                    rhs = AP(a2_ap.tensor, a2_ap.offset + off,
                             [[a2_ps, 128], [js, 2], [40, 16], [1, 32]])
                    lhsT = w_sb[2][:, 256 * p:256 * (p + 1)].rearrange(
                        "p (j c) -> p j c", j=2)
                    nc.tensor.matmul(pt[:, :, :], lhsT, rhs, start=(p == 0),
                                     stop=(p == 4), perf_mode=DR)
                trh = tmps.tile([128, 16, 16], f32, tag="trh", name="trh")
                nc.vector.tensor_reduce(
                    trh[:], pt[:].rearrange("p r (c two) -> p r c two",
                                            two=2), axis=AX.X, op=AL.max)
                pl = tmps.tile([128, 8, 16], f32, tag="pl", name="pl")
                vv = trh[:].rearrange("p (a two) c -> p a two c", two=2)
                nc.vector.scalar_tensor_tensor(
                    pl[:], vv[:, :, 0, :], 0.0, vv[:, :, 1, :], AL.max,
                    AL.max)
                nc.scalar.activation(a3[:, b, 1 + 8 * h:9 + 8 * h, 1:17],
                                     pl[:], AF.Sign, bias=b2_ap, scale=s2_ap)

        # interleave conv1 and L2 per image (2-image lead)
        conv1_emit(0)
        conv1_emit(1)
        for b in range(B):
            if b + 2 < B:
                conv1_emit(b + 2)
            l2_emit(b)

        # small consts needed only by the dense tail; sync queue is free now
        nc.sync.dma_start(dwp_sb[:], d_dwp)
        nc.sync.dma_start(db_sb[:], d_db)

        # deferred guard memsets (deadlines: L4's Sign ~95us, L5 ~115us)
        memset_wide(a5)
        memset_wide(a6)
        # a6 group1 partitions [ng1..128) are never written by L5's ACT:
        # zero the whole aligned tail (alive interiors rewritten by ACT).
        # The two ones-repr planes at g1 partitions ng1-2..ng1-1 are produced
        # by the ACT itself (zero weights, scale=0, bias=1 -> Sign(+1)).
        nc.gpsimd.memset(a6[96:128, 1, 0:_S5], 0.0)

        # ---- L3: tap-paired DR (5 passes), per-image; Sign reads PSUM -----
        a3_ap = a3[:]
        a3_ps = a3_ap.ap[0][0]
        for go in range(2):
            s3_ap, b3_ap = sb(4 + 2 * go), sb(5 + 2 * go)
            for b in range(B):
                pt = psum_pool.tile([128, 16, 16], f32, tag="cps", name="cps")
                for p in range(5):
                    if p < 3:
                        off, js = b * 480 + p, 48
                    elif p == 3:       # (1,0)+(1,1): in-plane j-stride 1
                        off, js = b * 480 + 24, 1
                    else:
                        off, js = b * 480 + 24 + 2, 48
                    rhs = AP(a3_ap.tensor, a3_ap.offset + off,
                             [[a3_ps, 128], [js, 2], [24, 16], [1, 16]])
                    lhsT = w_sb[3][:, go * 1280 + 256 * p:
                                   go * 1280 + 256 * (p + 1)].rearrange(
                        "p (j c) -> p j c", j=2)
                    nc.tensor.matmul(pt[:, :, :], lhsT, rhs, start=(p == 0),
                                     stop=(p == 4), perf_mode=DR)
                nc.scalar.activation(a4[:, go, b, 1:17, 1:17], pt[:, :, :],
                                     AF.Sign, bias=b3_ap, scale=s3_ap)

        # ---- L4: channel-paired DR, per-image, 2x2 pool -------------------
        a5_ap = a5[:]
        a5_ps = a5_ap.ap[0][0]
        for go in range(2):
            s4_ap, b4_ap = sb(8 + 2 * go), sb(9 + 2 * go)
            for b in range(B):
                pt = psum_pool.tile([128, 16, 16], f32, tag="cps", name="cps")
                for k in range(9):
                    dy, dx = k // 3, k % 3
                    col = (go * 9 + k) * 256
                    lhsT = w_sb[4][:, col:col + 256].rearrange(
                        "p (j c) -> p j c", j=2)
                    rhs = a4[:, :, b, dy:dy + 16, dx:dx + 16]
                    nc.tensor.matmul(pt[:, :, :], lhsT, rhs, start=(k == 0),
                                     stop=(k == 8), perf_mode=DR)
                trh = tmps.tile([128, 16, 8], f32, tag="trh4", name="trh4")
                nc.vector.tensor_reduce(
                    trh[:], pt[:].rearrange("p r (c two) -> p r c two",
                                            two=2), axis=AX.X, op=AL.max)
                pl = tmps.tile([128, 8, 8], f32, tag="pl4", name="pl4")
                vv = trh[:].rearrange("p (a two) c -> p a two c", two=2)
                nc.vector.scalar_tensor_tensor(
                    pl[:], vv[:, :, 0, :], 0.0, vv[:, :, 1, :], AL.max,
                    AL.max)
                dest = AP(a5_ap.tensor, a5_ap.offset + go * _S5 + _G5
                          + _WP5 + 1 + 9 * b,
                          [[a5_ps, 128], [_WP5, 8], [1, 8]])
                nc.scalar.activation(dest, pl[:], AF.Sign, bias=b4_ap,
                                     scale=s4_ap)

        # ---- L5: channel-paired DR on wide layout, 2 packed out groups ----
        a5f = a5[:]
        a6_ap = a6[:]
        a6_ps = a6_ap.ap[0][0]
        for go in range(2):
            # go1 includes the ones-repr as its last (zero-weight) column
            ncol = 128 if go == 0 else ng1
            s5_ap = bn_sb[0:ncol, 12 + go:13 + go]
            b5_ap = bn_sb[0:ncol, 14 + go:15 + go]
            for (r0, nr) in ((1, 3), (4, 3), (7, 2)):
                N = nr * _WP5
                pt = psum_pool.tile([ncol, N], f32, tag="cps", name="cps")
                for k in range(9):
                    dy, dx = k // 3, k % 3
                    off = _G5 + (r0 - 1 + dy) * _WP5 + dx - 1
                    lhsT = w_sb[5][:, (go * 9 + k) * 256:
                                   (go * 9 + k) * 256 + 256].rearrange(
                        "p (j c) -> p j c", j=2)[:, :, 0:ncol]
                    nc.tensor.matmul(pt[:], lhsT, a5f[:, :, off:off + N],
                                     start=(k == 0), stop=(k == 8),
                                     perf_mode=DR)
                src = AP(pt[:].tensor, pt[:].offset + 1,
                         [[pt[:].ap[0][0], ncol], [_WP5, nr], [9, 16], [1, 8]])
                dest = AP(a6_ap.tensor, a6_ap.offset + go * _S5 + _G5
                          + r0 * _WP5 + 1,
                          [[a6_ps, ncol], [_WP5, nr], [9, 16], [1, 8]])
                nc.scalar.activation(dest, src, AF.Sign, bias=b5_ap,
                                     scale=s5_ap)

        # ---- L6: channel-paired DR (9 passes), 2x2 pool, raw pool out -----
        # bn6 is folded into the dense weights; h6 holds raw pooled values.
        # The dense bias seeds the accumulators up front (start=True) so the
        # tail after the last h6 chunk is just the softmax chain.
        ptds = []
        for p in range(2):
            ptd = psum_d.tile([128, 10], f32, tag="dps", name="dps")
            nc.tensor.matmul(ptd[:, :], ones_sb[0:1, :], db_sb[0:1, :],
                             start=True, stop=False)
            ptds.append(ptd)
        for go in range(4):
            for rp in range(4):
                r0 = 1 + 2 * rp
                pt = psum_pool.tile([128, 2, _WP5], f32, tag="cps",
                                    name="cps")
                for k in range(9):
                    col = ((go * 9 + k)) * 256
                    lhsT = w_sb[6][:, col:col + 256].rearrange(
                        "p (j c) -> p j c", j=2)
                    off = _G5 + (r0 - 1 + k // 3) * _WP5 + (k % 3) - 1
                    nc.tensor.matmul(pt[:].rearrange("p a b -> p (a b)"),
                                     lhsT, a6_ap[:, :, off:off + 2 * _WP5],
                                     start=(k == 0), stop=(k == 8),
                                     perf_mode=DR)
                # relu-copy only the valid cols PSUM->SBUF (single PSUM
                # input per DVE op), then pool pairs in SBUF
                tv = pt[:, :, 1:145].rearrange("p r (i x) -> p r i x", x=9)
                ts = tmps.tile([128, 2, 16, 8], f32, tag="ts6", name="ts6")
                nc.vector.tensor_scalar(ts[:], tv[:, :, :, 0:8], 0.0, None,
                                        AL.max)
                tsp = ts[:].rearrange("p r i (u two) -> p r i u two", two=2)
                th = tmps.tile([128, 2, 16, 4], f32, tag="th6", name="th6")
                nc.vector.tensor_tensor(th[:], tsp[:, :, :, :, 0],
                                        tsp[:, :, :, :, 1], op=AL.max)
                nc.vector.tensor_tensor(h6[go][:, :, rp, :], th[:, 0],
                                        th[:, 1], op=AL.max)
            # h6[go] complete -> fold into the dense accumulation now
            for p in range(2):
                lhsT = h6[go][:, 8 * p:8 * p + 8, :, :]
                rhs = dwp_sb[:, go * 10:(go + 1) * 10]
                nc.tensor.matmul(ptds[p][:, :], lhsT, rhs,
                                 start=False, stop=(go == 3))

        # ---- softmax ------------------------------------------------------
        for p in range(2):
            ptd = ptds[p]
            mx = small.tile([128, 1], f32, tag="mx", name="mx")
            nc.vector.tensor_reduce(mx[:], ptd[:, :], axis=AX.X, op=AL.max,
                                    negate=True)
            e = small.tile([128, 10], f32, tag="e", name="e")
            ssum = small.tile([128, 1], f32, tag="ssum", name="ssum")
            nc.scalar.activation(e[:], ptd[:, :], AF.Exp, bias=mx[:],
                                 scale=1.0, accum_out=ssum[:])
            rcp = small.tile([128, 1], f32, tag="rcp", name="rcp")
            nc.vector.reciprocal(rcp[:], ssum[:])
            o = small.tile([128, 10], f32, tag="o", name="o")
            nc.vector.tensor_scalar(o[:], e[:], rcp[:], None, AL.mult)
            nc.sync.dma_start(d_out[128 * p:128 * (p + 1), :], o[:])

    nc.compile()
    _prog_cache[key] = nc
    return nc


# --------------------------------------------------------------------------
# host-side input packing
# --------------------------------------------------------------------------

def _trick_vectors(scale, bias):
    """sign(s*relu(z)+b) == sign(s*z+b) if b<0 else const +1 (scale=0,b=1)."""
    s = np.asarray(scale, np.float32).copy()
    bb = np.asarray(bias, np.float32).copy()
    dead = bb >= 0
    s[dead] = 0.0
    bb[dead] = 1.0
    return s, bb


def _pack_shared(inputs):
    f32 = np.float32
    # conv1 fp16 hi/lo: rows [w_hi(27); w_hi(27); w_lo(27)] x 128 outs
    w1 = np.asarray(inputs["w1"], f32).reshape(27, 128)
    w1_hi = w1.astype(_F16)
    w1_lo = (w1 - w1_hi.astype(f32)).astype(_F16)
    w1p = np.zeros((81, 128), _F16)
    w1p[0:27] = w1_hi
    w1p[27:54] = w1_hi
    w1p[54:81] = w1_lo

    b1 = np.asarray(inputs["b1"], f32)
    s1 = np.asarray(inputs["bn1_scale"], f32)
    b1p = np.asarray(inputs["bn1_bias"], f32)

    bnv = np.zeros((128, 16), f32)
    # conv1 Sign: alive: scale=s1, bias=s1*b1+b1'; dead(b1'>=0): (0, 1)
    sc = s1.copy()
    bc = s1 * b1 + b1p
    dead1 = b1p >= 0
    sc[dead1] = 0.0
    bc[dead1] = 1.0
    bnv[:, 0] = sc
    bnv[:, 1] = bc
    bnv[:, 2], bnv[:, 3] = _trick_vectors(inputs["bn2_scale"],
                                          inputs["bn2_bias"])
    for l, c0 in ((3, 4), (4, 8)):
        s, bb = _trick_vectors(inputs[f"bn{l}_scale"], inputs[f"bn{l}_bias"])
        for g in range(2):
            bnv[:, c0 + 2 * g] = s[128 * g:128 * (g + 1)]
            bnv[:, c0 + 2 * g + 1] = bb[128 * g:128 * (g + 1)]

    # bn5: packed alive channels
    s5 = np.asarray(inputs["bn5_scale"], f32)
    b5 = np.asarray(inputs["bn5_bias"], f32)
    alive5 = np.where(b5 < 0)[0]
    dead5 = np.where(b5 >= 0)[0]
    na5 = alive5.size
    ng1 = na5 + 2 - 128
    bnv[0:128, 12] = s5[alive5[0:128]]
    bnv[0:128, 14] = b5[alive5[0:128]]
    bnv[0:ng1 - 2, 13] = s5[alive5[128:na5]]
    bnv[0:ng1 - 2, 15] = b5[alive5[128:na5]]
    # two ones-repr channels: scale=0, bias=1 -> ACT writes constant +1
    bnv[ng1 - 2:ng1, 13] = 0.0
    bnv[ng1 - 2:ng1, 15] = 1.0

    wbs = {}
    # L2/L3 tap pairs: p<3 -> (tap(0,p), tap(2,p)); p=3 -> (tap(1,0),
    # tap(1,1)) via shifted plane; p=4 -> (tap(1,2), zeros)
    for l, Go in ((2, 1), (3, 2)):
        w = np.sign(np.asarray(inputs[f"w{l}"], f32)).astype(_F8)
        blob = np.zeros((128, _WCOLS[l]), _F8)
        for go in range(Go):
            wg = w[:, :, :, go * 128:(go + 1) * 128]   # [3,3,128,128]
            for p in range(5):
                base = go * 1280 + p * 256
                if p < 3:
                    blob[:, base:base + 128] = wg[0, p]
                    blob[:, base + 128:base + 256] = wg[2, p]
                elif p == 3:
                    blob[:, base:base + 128] = wg[1, 0]
                    blob[:, base + 128:base + 256] = wg[1, 1]
                else:
                    blob[:, base:base + 128] = wg[1, 2]
        wbs[l] = blob
    # L4: channel-pair per tap: [go][k][j=2 ch-group]
    w = np.sign(np.asarray(inputs["w4"], f32)).astype(_F8)
    blob = np.empty((128, _WCOLS[4]), _F8)
    for go in range(2):
        for k in range(9):
            base = (go * 9 + k) * 256
            for j in range(2):
                blob[:, base + j * 128:base + (j + 1) * 128] = \
                    w[k // 3, k % 3, j * 128:(j + 1) * 128,
                      go * 128:(go + 1) * 128]
    wbs[4] = blob
    # L5: channel-pair per tap, cols = packed alive5 outputs
    w = np.sign(np.asarray(inputs["w5"], f32)).astype(np.float32)
    blob = np.zeros((128, _WCOLS[5]), _F8)
    for go in range(2):
        cols = alive5[0:128] if go == 0 else alive5[128:na5]
        for k in range(9):
            base = (go * 9 + k) * 256
            for j in range(2):
                blob[:, base + j * 128:base + j * 128 + cols.size] = \
                    w[k // 3, k % 3, j * 128:(j + 1) * 128, :][:, cols
                                                              ].astype(_F8)
    wbs[5] = blob
    # L6: rows = packed alive5 input pairs (g0: alive5[0:128], g1:
    # alive5[128:na5] + two repr rows r/16q + zero pad), cols = all 512 outs
    w = np.sign(np.asarray(inputs["w6"], f32)).astype(np.float32)
    wsum = w[:, :, dead5, :].sum(axis=2)          # [3,3,512]
    wq = np.rint(wsum / 16.0)
    wr = wsum - 16.0 * wq
    assert np.abs(wq).max() <= 16 and np.abs(wr).max() <= 8
    blob = np.zeros((128, _WCOLS[6]), _F8)
    for go in range(4):
        oc = slice(go * 128, (go + 1) * 128)
        for k in range(9):
            base = (go * 9 + k) * 256
            blob[:, base:base + 128] = \
                w[k // 3, k % 3, alive5[0:128], oc].astype(_F8)
            blob[0:ng1 - 2, base + 128:base + 256] = \
                w[k // 3, k % 3, alive5[128:na5], oc].astype(_F8)
            blob[ng1 - 2, base + 128:base + 256] = \
                wr[k // 3, k % 3, oc].astype(_F8)
            blob[ng1 - 1, base + 128:base + 256] = \
                (16.0 * wq[k // 3, k % 3, oc]).astype(_F8)
    wbs[6] = blob

    # dense with bn6 folded: dwp'[c] = s6[c]*dw[c], db' = db + b6 @ dw
    dw = np.asarray(inputs["dense_w"], f32)
    s6 = np.asarray(inputs["bn6_scale"], f32)
    b6 = np.asarray(inputs["bn6_bias"], f32)
    dws = dw * s6[:, None]
    dwp = dws.reshape(4, 128, 10).transpose(1, 0, 2).reshape(128, 40).copy()
    db = (np.asarray(inputs["dense_b"], f32) + b6 @ dw).reshape(1, 10).copy()
    return w1p, bnv, wbs, dwp, db, na5


def _pack_xcol(x16):
    """[16,32,32,3] f32 -> [81,16384] fp16 hi/lo im2col."""
    xp = np.zeros((B, 34, 34, 3), np.float32)
    xp[:, 1:33, 1:33, :] = x16
    cols = np.empty((27, B, 32, 32), np.float32)
    for ky in range(3):
        for kx in range(3):
            for ci in range(3):
                r = (ky * 3 + kx) * 3 + ci
                cols[r] = xp[:, ky:ky + 32, kx:kx + 32, ci]
    cols = cols.reshape(27, B * 1024)
    hi = cols.astype(_F16)
    lo = (cols - hi.astype(np.float32)).astype(_F16)
    xcol = np.empty((81, 16384), _F16)
    xcol[0:27] = hi
    xcol[27:54] = lo
    xcol[54:81] = hi
    return xcol


def _make_in_maps(inputs):
    w1p, bnv, wbs, dwp, db, na5 = _pack_shared(inputs)
    x = np.asarray(inputs["x"], np.float32)
    in_maps = []
    for c in range(N_CORES):
        m = {"xcol": _pack_xcol(x[B * c:B * (c + 1)]),
             "w1p": w1p, "bnv": bnv, "dwp": dwp, "db": db}
        for l in wbs:
            m[f"wb{l}"] = wbs[l]
        in_maps.append(m)
    return in_maps, na5


def _run(inputs, trace=False):
    """Returns (output [128,4,4,10] f32, BassKernelResults)."""
    in_maps, na5 = _make_in_maps(inputs)
    nc = _build_program(na5)
    from concourse.bass_utils import run_bass_kernel_spmd
    res = run_bass_kernel_spmd(nc, in_maps, list(range(N_CORES)), trace=trace)
    outs = [res.results[c]["out"].reshape(B, 4, 4, 10)
            for c in range(N_CORES)]
    return np.concatenate(outs, axis=0), res


def kernel(**inputs):
    out, _ = _run(inputs)
    return out
